# revision 1
# baseline (speedup 1.0000x reference)
"""Trainium2 Bass kernel for GQA MHA with causal depthwise conv + rotary.

Sharding: 8 cores = 2 batches x 4 head-groups. Each core (b, g) computes
q heads 4g..4g+3 and kv head g for batch b (tensor-parallel over heads,
data-parallel over batch; GQA repeat stays core-local). The out-projection
is row-sharded over head groups, producing partial [S, E] sums per core
that are reduced on the host during unshard (standard row-parallel
unshard), plus b_out.

Device layout choices:
  - qkv computed in [c, s] layout (channels on partitions) so the depthwise
    conv along s is a free-dim shifted-window op and rotary is elementwise.
  - attention uses the "scores transposed" layout: scoresT[k, q] tiles from
    matmul(lhsT=kT, rhs=qT); exp on ACT; softmax denominator via a
    ones-vector matmul (column sum); ctxT[d, q] = v_sd.T @ expT. No max
    subtraction is needed: logits here are O(0.1), exp cannot overflow.
  - matmul inputs in bf16 (4x faster PE than fp32), fp32 PSUM accumulate.
"""

import numpy as np
import ml_dtypes

E = 2048
H = 16
HKV = 4
D = 128
DCONV = 4
ROT_BASE = 10000.0
B, S = 2, 2048
QKV_DIM = D * (H + 2 * HKV)   # 3072
N_CORES = 8
HL = 4                         # local q heads per core
CL = (HL + 2) * D              # 768 local qkv channels
NCT = CL // 128                # 6 local c-tiles (4 q heads, 1 k, 1 v)
SCW = 512                      # s-chunk width
NSC = S // SCW                 # 4
NEO = E // 128                 # 16 contraction chunks for the input GEMM
NST = S // 128                 # 16 s-tiles
BF = ml_dtypes.bfloat16
SCALE = 1.0 / float(np.sqrt(D))

_cache: dict = {}


def _build_program():
    import concourse.bacc as bacc
    import concourse.tile as tile
    import concourse.mybir as mybir
    from concourse.bass import ts

    fp32 = mybir.dt.float32
    bf16 = mybir.dt.bfloat16

    nc = bacc.Bacc("TRN2", target_bir_lowering=False, debug=False)

    # ---- device I/O ----
    xT = nc.dram_tensor("xT", [E, S], bf16, kind="ExternalInput")
    win = nc.dram_tensor("win", [NCT, 128, NEO, 128], bf16, kind="ExternalInput")
    wout = nc.dram_tensor("wout", [HL * D, E], bf16, kind="ExternalInput")
    binv = nc.dram_tensor("binv", [128, NCT], fp32, kind="ExternalInput")
    convw = nc.dram_tensor("convw", [128, NCT, DCONV], fp32, kind="ExternalInput")
    convb = nc.dram_tensor("convb", [128, NCT], fp32, kind="ExternalInput")
    cos2 = nc.dram_tensor("cos2", [128, S], bf16, kind="ExternalInput")
    sin2 = nc.dram_tensor("sin2", [128, S], bf16, kind="ExternalInput")
    masks = nc.dram_tensor("masks", [128, 4, SCW], bf16, kind="ExternalInput")
    ident = nc.dram_tensor("ident", [128, 128], bf16, kind="ExternalInput")
    out_p = nc.dram_tensor("out_p", [S, E], fp32, kind="ExternalOutput")

    CONV_ORDER = (4, 0, 5, 1, 2, 3)   # k, q0, v first: attention starts early
    LA = 4                            # score-pipeline lookahead (PE FIFO depth)

    with tile.TileContext(nc) as tc:
        with (
            tc.tile_pool(name="const", bufs=1) as cpool,
            tc.tile_pool(name="xt", bufs=2) as xpool,
            tc.tile_pool(name="qkvpad", bufs=1) as padpool,
            tc.tile_pool(name="ctmp", bufs=2) as ctmp,
            tc.tile_pool(name="rtmp", bufs=2) as rtmp,
            tc.tile_pool(name="qk", bufs=NCT) as qkpool,
            tc.tile_pool(name="vsd", bufs=1) as vpool,
            tc.tile_pool(name="exp", bufs=6) as epool,
            tc.tile_pool(name="ctx", bufs=HL) as ctxpool,
            tc.tile_pool(name="rec", bufs=2) as rpool,
            tc.tile_pool(name="outsb", bufs=8) as opool,
            tc.tile_pool(name="psS", bufs=2, space="PSUM") as psS,
            tc.tile_pool(name="psMM", bufs=2, space="PSUM") as psMM,
            tc.tile_pool(name="psC", bufs=2, space="PSUM") as psC,
        ):
            # ---- constants; DMA emission order == need order ----
            ones_t = cpool.tile([128, 128], bf16)
            nc.vector.memset(ones_t[:], 1.0)
            zb_t = cpool.tile([128, 1], fp32)
            nc.vector.memset(zb_t[:], 0.0)

            win_t = cpool.tile([128, NEO, CL], bf16)

            xt_tiles = [None] * NSC
            xT_r = xT[:].rearrange("(eo p) s -> p eo s", p=128)

            def load_xt(sc):
                xt = xpool.tile([128, NEO, SCW], bf16, tag="xt", name=f"xt{sc}")
                for qtr in range(4):   # quarter DMAs: first matmul can start early
                    nc.sync.dma_start(
                        xt[:, ts(qtr, 4), :],
                        xT_r[:, ts(qtr, 4), ts(sc, SCW)],
                    )
                xt_tiles[sc] = xt

            xt0 = xpool.tile([128, NEO, SCW], bf16, tag="xt", name="xt0")
            for qtr in range(4):
                nc.sync.dma_start(
                    win_t[:, ts(qtr, 4), ts(CONV_ORDER[0], 128)],
                    win[CONV_ORDER[0], :, ts(qtr, 4), :],
                )
                nc.sync.dma_start(
                    xt0[:, ts(qtr, 4), :],
                    xT_r[:, ts(qtr, 4), ts(0, SCW)],
                )
            xt_tiles[0] = xt0
            binv_t = cpool.tile([128, NCT], fp32)
            nc.sync.dma_start(binv_t[:], binv[:])
            convw_t = cpool.tile([128, NCT, DCONV], fp32)
            nc.sync.dma_start(convw_t[:], convw[:])
            convb_t = cpool.tile([128, NCT], fp32)
            nc.sync.dma_start(convb_t[:], convb[:])
            for ct in CONV_ORDER[1:]:
                nc.sync.dma_start(win_t[:, :, ts(ct, 128)], win[ct])
            cos_t = cpool.tile([128, S], bf16)
            nc.sync.dma_start(cos_t[:], cos2[:])
            sin_t = cpool.tile([128, S], bf16)
            nc.sync.dma_start(sin_t[:], sin2[:])
            id_t = cpool.tile([128, 128], bf16)
            nc.sync.dma_start(id_t[:], ident[:])
            mask_t = cpool.tile([128, 4, SCW], bf16)
            nc.sync.dma_start(mask_t[:], masks[:])
            wout_t = cpool.tile([128, HL, E], bf16)
            nc.sync.dma_start(wout_t[:], wout[:].rearrange("(co p) e -> p co e", p=128))

            qkv_pad = padpool.tile([128, NCT, S + DCONV - 1], bf16)
            nc.vector.memset(qkv_pad[:, :, 0 : DCONV - 1], 0.0)

            qcb = [None] * NCT
            for ct in range(NCT):
                qcb[ct] = qkpool.tile([128, S], bf16, tag="qcb", name=f"qcb{ct}")
            v_sd = vpool.tile([128, NST, 128], bf16)
            ctxT = [None] * HL
            for h in range(HL):
                ctxT[h] = ctxpool.tile([128, S], bf16, tag="ctxT", name=f"ctxT{h}")

            def gemm_chunk(sc):
                xt = xt_tiles[sc]
                for ct in CONV_ORDER:
                    ps = psMM.tile([128, SCW], fp32, tag="mm", name=f"g{sc}_{ct}")
                    for eo in range(NEO):
                        nc.tensor.matmul(
                            ps[:],
                            win_t[:, eo, ts(ct, 128)],
                            xt[:, eo, :],
                            start=(eo == 0),
                            stop=(eo == NEO - 1),
                        )
                    nc.scalar.activation(
                        qkv_pad[:, ct, DCONV - 1 + sc * SCW : DCONV - 1 + (sc + 1) * SCW],
                        ps[:],
                        mybir.ActivationFunctionType.Identity,
                        bias=binv_t[:, ct : ct + 1],
                    )
            def conv_rot_chunk(sc):
                for ct in CONV_ORDER:
                    # depthwise causal conv taps via fused (in0*w + acc) ops
                    t0 = ctmp.tile([128, SCW], fp32, tag="ctmp", name=f"t0_{sc}_{ct}")
                    nc.vector.tensor_scalar(
                        t0[:], qkv_pad[:, ct, sc * SCW : sc * SCW + SCW],
                        convw_t[:, ct, 0:1], convb_t[:, ct : ct + 1],
                        mybir.AluOpType.mult, mybir.AluOpType.add,
                    )
                    t1 = ctmp.tile([128, SCW], fp32, tag="ctmp", name=f"t1_{sc}_{ct}")
                    nc.vector.scalar_tensor_tensor(
                        t1[:], qkv_pad[:, ct, sc * SCW + 1 : sc * SCW + 1 + SCW],
                        convw_t[:, ct, 1:2], t0[:],
                        mybir.AluOpType.mult, mybir.AluOpType.add,
                    )
                    t2 = ctmp.tile([128, SCW], fp32, tag="ctmp", name=f"t2_{sc}_{ct}")
                    nc.vector.scalar_tensor_tensor(
                        t2[:], qkv_pad[:, ct, sc * SCW + 2 : sc * SCW + 2 + SCW],
                        convw_t[:, ct, 2:3], t1[:],
                        mybir.AluOpType.mult, mybir.AluOpType.add,
                    )
                    nc.vector.scalar_tensor_tensor(
                        qcb[ct][:, ts(sc, SCW)],
                        qkv_pad[:, ct, sc * SCW + 3 : sc * SCW + 3 + SCW],
                        convw_t[:, ct, 3:4], t2[:],
                        mybir.AluOpType.mult, mybir.AluOpType.add,
                    )
                    if ct == 5:
                        for sti in range(4):
                            st = 4 * sc + sti
                            pvt = psMM.tile([128, 128], bf16, tag="mm", name=f"vt{st}")
                            nc.tensor.transpose(pvt[:], qcb[5][:, ts(st, 128)], id_t[:])
                            nc.vector.tensor_copy(v_sd[:, st, :], pvt[:])
                    else:
                        # rotary in place; half-swap via cross-partition DVE copies
                        sl = ts(sc, SCW)
                        qsw = rtmp.tile([128, SCW], bf16, tag="qsw", name=f"qsw{sc}_{ct}")
                        nc.vector.tensor_copy(qsw[0:64, :], qcb[ct][64:128, sl])
                        nc.vector.tensor_copy(qsw[64:128, :], qcb[ct][0:64, sl])
                        m1 = rtmp.tile([128, SCW], bf16, tag="rtmp", name=f"m1_{sc}_{ct}")
                        nc.vector.tensor_mul(m1[:], qcb[ct][:, sl], cos_t[:, sl])
                        m2 = rtmp.tile([128, SCW], bf16, tag="rtmp", name=f"m2_{sc}_{ct}")
                        nc.vector.tensor_mul(m2[:], qsw[:], sin_t[:, sl])
                        nc.vector.tensor_add(qcb[ct][:, sl], m1[:], m2[:])

            attn_state = {}

            def attn_prep(qc):
                nkt = 4 * (qc + 1)
                kt_order = list(range(nkt - 4, nkt)) + list(range(nkt - 4))
                pairs = [(kt_order[2 * j], kt_order[2 * j + 1]) for j in range(nkt // 2)]
                flat = [(h, j) for h in range(HL) for j in range(len(pairs))]
                ets = {}

                def scores_pair(h, j):
                    ka, kb = pairs[j]
                    scps = psS.tile([128, 2, SCW], fp32, tag="sc", name=f"sc{h}_{qc}_{j}")
                    nc.tensor.matmul(
                        scps[:, 0, :], qcb[4][:, ts(ka, 128)],
                        qcb[h][:, ts(qc, SCW)], start=True, stop=True,
                    )
                    nc.tensor.matmul(
                        scps[:, 1, :], qcb[4][:, ts(kb, 128)],
                        qcb[h][:, ts(qc, SCW)], start=True, stop=True,
                    )
                    et = epool.tile([128, 2, SCW], bf16, tag="exp", name=f"e{h}_{qc}_{j}")
                    nc.scalar.activation(
                        et[:], scps[:],
                        mybir.ActivationFunctionType.Exp,
                        bias=zb_t[:, 0:1], scale=SCALE,
                    )
                    ja = pairs[j][0] - (nkt - 4)
                    if ja >= 0:
                        nc.vector.tensor_mul(et[:], et[:], mask_t[:, ja : ja + 2, :])
                    ets[h, j] = et

                return dict(pairs=pairs, flat=flat, ets=ets, scores_pair=scores_pair)

            LAP = 2

            def attn_prefill(qc):
                st = attn_state[qc] = attn_prep(qc)
                for idx in range(min(LAP, len(st["flat"]))):
                    st["scores_pair"](*st["flat"][idx])

            def attn_body(qc):
                st = attn_state.pop(qc)
                pairs, flat, ets, scores_pair = (
                    st["pairs"], st["flat"], st["ets"], st["scores_pair"])
                npair = len(pairs)
                cps = {}
                sps = {}
                for idx, (h, j) in enumerate(flat):
                    if idx + LAP < len(flat):
                        scores_pair(*flat[idx + LAP])
                    if j == 0:
                        cps[h] = psC.tile([128, SCW], fp32, tag="ctx", name=f"c{h}_{qc}")
                        sps[h] = psMM.tile([128, SCW], fp32, tag="mm", name=f"s{h}_{qc}")
                    ka, kb = pairs[j]
                    et = ets.pop((h, j))
                    first, last = (j == 0), (j == npair - 1)
                    nc.tensor.matmul(
                        cps[h][:], v_sd[:, ka, :], et[:, 0, :],
                        start=first, stop=False,
                    )
                    nc.tensor.matmul(
                        sps[h][:], ones_t[:], et[:, 0, :],
                        start=first, stop=False,
                    )
                    nc.tensor.matmul(
                        cps[h][:], v_sd[:, kb, :], et[:, 1, :],
                        start=False, stop=last,
                    )
                    nc.tensor.matmul(
                        sps[h][:], ones_t[:], et[:, 1, :],
                        start=False, stop=last,
                    )
                    if last:
                        rec = rpool.tile([1, SCW], fp32, tag="rec", name=f"r{h}_{qc}")
                        nc.vector.reciprocal_approx_fast(rec[:], sps[h][0:1, :])
                        recb = rpool.tile([128, SCW], fp32, tag="recb", name=f"rb{h}_{qc}")
                        nc.gpsimd.partition_broadcast(recb[:], rec[:])
                        nc.vector.tensor_mul(
                            ctxT[h][:, ts(qc, SCW)], cps[h][:], recb[:]
                        )

            def outproj_chunk(qc):
                dma_eng = nc.sync if qc == NSC - 1 else nc.gpsimd
                for sti in range(4):
                    st = qc * 4 + sti
                    for ec in range(NSC):
                        po = psC.tile([128, SCW], fp32, tag="ctx", name=f"o{st}_{ec}")
                        for h in range(HL):
                            nc.tensor.matmul(
                                po[:],
                                ctxT[h][:, ts(st, 128)],
                                wout_t[:, h, ts(ec, SCW)],
                                start=(h == 0), stop=(h == HL - 1),
                            )
                        ob = opool.tile([128, SCW], fp32, tag="ob", name=f"ob{st}_{ec}")
                        nc.scalar.copy(ob[:], po[:])
                        dma_eng.dma_start(out_p[ts(st, 128), ts(ec, SCW)], ob[:])

            # ---- fused main loop, attention one chunk behind the GEMM:
            # conv/rot DVE work for chunk sc hides under attention(sc-1) PE work
            for sc in range(NSC):
                if sc + 1 < NSC:
                    load_xt(sc + 1)
                if sc > 0:
                    attn_prefill(sc - 1)
                gemm_chunk(sc)
                if sc > 0:
                    attn_body(sc - 1)
                    outproj_chunk(sc - 1)
                conv_rot_chunk(sc)
            attn_prefill(NSC - 1)
            attn_body(NSC - 1)
            outproj_chunk(NSC - 1)

    nc.compile()
    return nc


def _host_prep():
    """Precompute per-core-independent constant arrays."""
    inv_freq = 1.0 / (ROT_BASE ** (np.arange(0, D, 2, dtype=np.float32) / D))
    t = np.arange(S, dtype=np.float32)
    freqs = np.outer(t, inv_freq)                       # [S, 64]
    cos = np.cos(freqs).T                               # [64, S]
    sin = np.sin(freqs).T
    cos2 = np.concatenate([cos, cos], axis=0).astype(BF)     # [128, S]
    sin2 = np.concatenate([-sin, sin], axis=0).astype(BF)
    k = np.arange(128)[:, None]
    q = np.arange(SCW)[None, :]
    masks = np.stack(
        [(k + 128 * j <= q).astype(np.float32) for j in range(4)], axis=1
    ).astype(BF)                                        # [128, 4, 512]
    ident = np.eye(128, dtype=np.float32).astype(BF)
    return cos2, sin2, masks, ident


def _shard_inputs(x, W_in, b_in, conv_w, conv_b, W_out):
    cos2, sin2, masks, ident = _host_prep()
    xT = [np.ascontiguousarray(np.asarray(x[b]).T).astype(BF) for b in range(B)]
    in_maps = []
    for core in range(N_CORES):
        b, g = divmod(core, 4)
        qcols = slice(g * HL * D, (g + 1) * HL * D)
        kcols = slice(H * D + g * D, H * D + (g + 1) * D)
        vcols = slice(H * D + HKV * D + g * D, H * D + HKV * D + (g + 1) * D)
        csel = np.r_[qcols, kcols, vcols]               # 768 channel indices
        win_s = np.ascontiguousarray(
            W_in[:, csel].reshape(NEO, 128, NCT, 128).transpose(2, 1, 0, 3)
        ).astype(BF)                                               # [6, 128, 16, 128]
        binv_s = np.ascontiguousarray(
            b_in[csel].reshape(NCT, 128).T).astype(np.float32)     # [128, 6]
        convw_s = np.ascontiguousarray(
            conv_w[csel].reshape(NCT, 128, DCONV).transpose(1, 0, 2)
        ).astype(np.float32)                                       # [128, 6, 4]
        convb_s = np.ascontiguousarray(
            conv_b[csel].reshape(NCT, 128).T).astype(np.float32)
        wout_s = np.ascontiguousarray(
            W_out[g * HL * D : (g + 1) * HL * D, :]).astype(BF)    # [512, E]
        in_maps.append({
            "xT": xT[b],
            "win": win_s,
            "wout": wout_s,
            "binv": binv_s,
            "convw": convw_s,
            "convb": convb_s,
            "cos2": cos2,
            "sin2": sin2,
            "masks": masks,
            "ident": ident,
        })
    return in_maps


def _get_nc():
    if "nc" not in _cache:
        _cache["nc"] = _build_program()
    return _cache["nc"]


def run(x, W_in, b_in, conv_w, conv_b, W_out, b_out, trace=False, **rb_kwargs):
    from concourse import bass_utils

    x = np.asarray(x, dtype=np.float32)
    W_in = np.asarray(W_in, dtype=np.float32)
    b_in = np.asarray(b_in, dtype=np.float32)
    conv_w = np.asarray(conv_w, dtype=np.float32)
    conv_b = np.asarray(conv_b, dtype=np.float32)
    W_out = np.asarray(W_out, dtype=np.float32)
    b_out = np.asarray(b_out, dtype=np.float32)

    nc = _get_nc()
    in_maps = _shard_inputs(x, W_in, b_in, conv_w, conv_b, W_out)
    res = bass_utils.run_bass_kernel_spmd(
        nc, in_maps, core_ids=list(range(N_CORES)), trace=trace, **rb_kwargs
    )
    partial = [res.results[c]["out_p"] for c in range(N_CORES)]
    out = np.empty((B, S, E), dtype=np.float32)
    for b in range(B):
        acc = partial[4 * b].astype(np.float64)
        for g in range(1, 4):
            acc += partial[4 * b + g]
        out[b] = (acc + b_out.astype(np.float64)).astype(np.float32)
    return out, res


def kernel(x, W_in, b_in, conv_w, conv_b, W_out, b_out):
    out, _ = run(x, W_in, b_in, conv_w, conv_b, W_out, b_out, trace=False)
    return out



# revision 3
# speedup vs baseline: 1.0686x; 1.0686x over previous
"""Trainium2 Bass kernel for GQA MHA with causal depthwise conv + rotary.

Sharding: 8 cores = 2 batches x 4 head-groups. Each core (b, g) computes
q heads 4g..4g+3 and kv head g for batch b (tensor-parallel over heads,
data-parallel over batch; GQA repeat stays core-local). The out-projection
is row-sharded over head groups, producing partial [S, E] sums per core
that are reduced on the host during unshard, plus b_out.

Device layout choices:
  - qkv computed in [c, s] layout (channels on partitions) so the depthwise
    conv along s is a free-dim shifted-window op and rotary is elementwise.
  - fp16 everywhere on the 16-bit path (same PE/DVE speed as bf16, 8x the
    mantissa); fp32 PSUM accumulate.
  - conv reads come from two per-chunk ring buffers (pad_e for taps 0/2,
    pad_o, stored shifted by one, for taps 1/3) so every DVE operand is
    4B-aligned and the fp16 2x perf mode engages.
  - attention uses the "scores transposed" layout: scoresT[k, q] tiles from
    matmul(lhsT=kT, rhs=qT); exp on ACT; ctxT[d, q] = v_sd.T @ expT. No max
    subtraction is needed: logits here are O(0.1), exp cannot overflow.
  - causal trim: for the 4 diagonal k-tiles of each q-chunk the scores/ctx/
    denominator matmuls only cover q >= k-tile start; the within-tile
    triangle is a single [128,128] mask multiply per diagonal tile.
  - softmax denominator: old (fully-causal) exp tiles are pre-summed on the
    DVE, so the ones-matmul column reduction contracts 1 merged tile + 4
    trimmed diagonal tiles instead of all k-tiles. The reciprocal runs on
    the full [128, 512] PSUM tile (all rows identical), so no partition
    broadcast is needed.
"""

import numpy as np
import ml_dtypes

E = 2048
H = 16
HKV = 4
D = 128
DCONV = 4
ROT_BASE = 10000.0
B, S = 2, 2048
QKV_DIM = D * (H + 2 * HKV)   # 3072
N_CORES = 8
HL = 4                         # local q heads per core
CL = (HL + 2) * D              # 768 local qkv channels
NCT = CL // 128                # 6 local c-tiles (4 q heads, 1 k, 1 v)
SCW = 512                      # s-chunk width
NSC = S // SCW                 # 4
NEO = E // 128                 # 16 contraction chunks for the input GEMM
NST = S // 128                 # 16 s-tiles
F16 = np.float16
SCALE = 1.0 / float(np.sqrt(D))
PADW = 516                     # per-chunk pad ring width (3/4 halo + 512)

_cache: dict = {}


def _build_program():
    import concourse.bacc as bacc
    import concourse.tile as tile
    import concourse.mybir as mybir
    from concourse.bass import ts

    fp32 = mybir.dt.float32
    f16 = mybir.dt.float16

    nc = bacc.Bacc("TRN2", target_bir_lowering=False, debug=False)

    # ---- device I/O ----
    xT = nc.dram_tensor("xT", [E, S], f16, kind="ExternalInput")
    win = nc.dram_tensor("win", [NCT, 128, NEO, 128], f16, kind="ExternalInput")
    wout = nc.dram_tensor("wout", [HL * D, E], f16, kind="ExternalInput")
    binv = nc.dram_tensor("binv", [128, NCT], fp32, kind="ExternalInput")
    convw = nc.dram_tensor("convw", [128, NCT, DCONV], fp32, kind="ExternalInput")
    convb = nc.dram_tensor("convb", [128, NCT], fp32, kind="ExternalInput")
    cos2 = nc.dram_tensor("cos2", [128, S], f16, kind="ExternalInput")
    sin2 = nc.dram_tensor("sin2", [128, S], f16, kind="ExternalInput")
    tri = nc.dram_tensor("tri", [128, 128], f16, kind="ExternalInput")
    ident = nc.dram_tensor("ident", [128, 128], f16, kind="ExternalInput")
    out_p = nc.dram_tensor("out_p", [S, E], f16, kind="ExternalOutput")

    CONV_ORDER = (4, 0, 5, 1, 2, 3)   # k, q0, v first: attention starts early
    LAP = 2                           # score-pipeline lookahead (pairs)

    with tile.TileContext(nc) as tc:
        with (
            tc.tile_pool(name="const", bufs=1) as cpool,
            tc.tile_pool(name="xt", bufs=2) as xpool,
            tc.tile_pool(name="qkvpad", bufs=1) as padpool,
            tc.tile_pool(name="ctmp", bufs=3) as ctmp,
            tc.tile_pool(name="rtmp", bufs=2) as rtmp,
            tc.tile_pool(name="qk", bufs=NCT) as qkpool,
            tc.tile_pool(name="vsd", bufs=1) as vpool,
            tc.tile_pool(name="exp", bufs=7) as epool,
            tc.tile_pool(name="den", bufs=2) as denpool,
            tc.tile_pool(name="dm", bufs=2) as dmpool,
            tc.tile_pool(name="ctx", bufs=HL) as ctxpool,
            tc.tile_pool(name="rec", bufs=2) as rpool,
            tc.tile_pool(name="outsb", bufs=6) as opool,
            tc.tile_pool(name="psS", bufs=2, space="PSUM") as psS,
            tc.tile_pool(name="psMM", bufs=2, space="PSUM") as psMM,
            tc.tile_pool(name="psC", bufs=2, space="PSUM") as psC,
        ):
            # ---- tiles + fine-grained startup DMA (first GEMM tile ASAP) ----
            win_t = cpool.tile([128, NEO, CL], f16)
            xt_tiles = [None] * NSC
            xT_r = xT[:].rearrange("(eo p) s -> p eo s", p=128)
            xt0 = xpool.tile([128, NEO, SCW], f16, tag="xt", name="xt0")
            ct0 = CONV_ORDER[0]

            # priority: exactly what matmul (sc=0, ct0, eo=0..3) needs, eo at
            # a time, then quarter-granularity for the rest.
            for eo in range(4):
                nc.sync.dma_start(
                    win_t[:, eo : eo + 1, ts(ct0, 128)], win[ct0, :, eo : eo + 1, :]
                )
                nc.sync.dma_start(xt0[:, eo : eo + 1, :], xT_r[:, eo : eo + 1, ts(0, SCW)])
            for qtr in range(1, 4):
                nc.sync.dma_start(
                    win_t[:, ts(qtr, 4), ts(ct0, 128)], win[ct0, :, ts(qtr, 4), :]
                )
                nc.sync.dma_start(xt0[:, ts(qtr, 4), :], xT_r[:, ts(qtr, 4), ts(0, SCW)])
            xt_tiles[0] = xt0

            binv_t = cpool.tile([128, NCT], fp32)
            nc.gpsimd.dma_start(binv_t[:], binv[:])
            convw_t = cpool.tile([128, NCT, DCONV], fp32)
            nc.gpsimd.dma_start(convw_t[:], convw[:])
            convb_t = cpool.tile([128, NCT], fp32)
            nc.gpsimd.dma_start(convb_t[:], convb[:])
            cos_t = cpool.tile([128, S], f16)
            nc.gpsimd.dma_start(cos_t[:], cos2[:])
            sin_t = cpool.tile([128, S], f16)
            nc.gpsimd.dma_start(sin_t[:], sin2[:])
            id_t = cpool.tile([128, 128], f16)
            nc.gpsimd.dma_start(id_t[:], ident[:])
            tri_t = cpool.tile([128, 128], f16)
            nc.gpsimd.dma_start(tri_t[:], tri[:])

            for ct in CONV_ORDER[1:]:
                nc.sync.dma_start(win_t[:, :, ts(ct, 128)], win[ct])
            wout_t = cpool.tile([128, HL, E], f16)
            nc.gpsimd.dma_start(
                wout_t[:], wout[:].rearrange("(co p) e -> p co e", p=128)
            )

            ones_t = cpool.tile([128, 128], f16)
            nc.vector.memset(ones_t[:], 1.0)
            zb_t = cpool.tile([128, 1], fp32)
            nc.vector.memset(zb_t[:], 0.0)

            def load_xt(sc):
                xt = xpool.tile([128, NEO, SCW], f16, tag="xt", name=f"xt{sc}")
                for qtr in range(4):
                    nc.sync.dma_start(
                        xt[:, ts(qtr, 4), :], xT_r[:, ts(qtr, 4), ts(sc, SCW)]
                    )
                xt_tiles[sc] = xt

            # per-chunk conv input rings: pad_e holds raw[s0-3+i] at index i
            # (taps 0/2 at even offsets); pad_o holds raw[s0-4+i] at index i
            # (taps 1/3 at even offsets). Double-buffered over chunks.
            pad_e = padpool.tile([128, NCT, 2, PADW], f16, name="pad_e")
            pad_o = padpool.tile([128, NCT, 2, PADW], f16, name="pad_o")
            nc.vector.memset(pad_e[:, :, 0, 0:3], 0.0)
            nc.vector.memset(pad_o[:, :, 0, 0:4], 0.0)

            qcb = [None] * NCT
            for ct in range(NCT):
                qcb[ct] = qkpool.tile([128, S], f16, tag="qcb", name=f"qcb{ct}")
            v_sd = vpool.tile([128, NST, 128], f16)
            ctxT = [None] * HL
            for h in range(HL):
                ctxT[h] = ctxpool.tile([128, S], f16, tag="ctxT", name=f"ctxT{h}")

            def gemm_chunk(sc):
                xt = xt_tiles[sc]
                buf = sc % 2
                for ct in CONV_ORDER:
                    ps = psMM.tile([128, SCW], fp32, tag="mm", name=f"g{sc}_{ct}")
                    for eo in range(NEO):
                        nc.tensor.matmul(
                            ps[:],
                            win_t[:, eo, ts(ct, 128)],
                            xt[:, eo, :],
                            start=(eo == 0),
                            stop=(eo == NEO - 1),
                        )
                    nc.scalar.activation(
                        pad_e[:, ct, buf, 3 : 3 + SCW], ps[:],
                        mybir.ActivationFunctionType.Identity,
                        bias=binv_t[:, ct : ct + 1],
                    )
                    nc.scalar.activation(
                        pad_o[:, ct, buf, 4 : 4 + SCW], ps[:],
                        mybir.ActivationFunctionType.Identity,
                        bias=binv_t[:, ct : ct + 1],
                    )

            def conv_rot_chunk(sc):
                buf = sc % 2
                for ct in CONV_ORDER:
                    if sc > 0:
                        nc.vector.tensor_copy(
                            pad_e[:, ct, buf, 0:3], pad_e[:, ct, 1 - buf, SCW : SCW + 3]
                        )
                        nc.vector.tensor_copy(
                            pad_o[:, ct, buf, 0:4], pad_o[:, ct, 1 - buf, SCW : SCW + 4]
                        )
                    # depthwise causal conv taps via fused (in0*w + acc) ops
                    t0 = ctmp.tile([128, SCW], f16, tag="ctmp", name=f"t0_{sc}_{ct}")
                    nc.vector.tensor_scalar(
                        t0[:], pad_e[:, ct, buf, 0:SCW],
                        convw_t[:, ct, 0:1], convb_t[:, ct : ct + 1],
                        mybir.AluOpType.mult, mybir.AluOpType.add,
                    )
                    t1 = ctmp.tile([128, SCW], f16, tag="ctmp", name=f"t1_{sc}_{ct}")
                    nc.vector.scalar_tensor_tensor(
                        t1[:], pad_o[:, ct, buf, 2 : 2 + SCW],
                        convw_t[:, ct, 1:2], t0[:],
                        mybir.AluOpType.mult, mybir.AluOpType.add,
                    )
                    t2 = ctmp.tile([128, SCW], f16, tag="ctmp", name=f"t2_{sc}_{ct}")
                    nc.vector.scalar_tensor_tensor(
                        t2[:], pad_e[:, ct, buf, 2 : 2 + SCW],
                        convw_t[:, ct, 2:3], t1[:],
                        mybir.AluOpType.mult, mybir.AluOpType.add,
                    )
                    nc.vector.scalar_tensor_tensor(
                        qcb[ct][:, ts(sc, SCW)], pad_o[:, ct, buf, 4 : 4 + SCW],
                        convw_t[:, ct, 3:4], t2[:],
                        mybir.AluOpType.mult, mybir.AluOpType.add,
                    )
                    if ct == 5:
                        for sti in range(4):
                            st = 4 * sc + sti
                            pvt = psMM.tile([128, 128], f16, tag="mm", name=f"vt{st}")
                            nc.tensor.transpose(pvt[:], qcb[5][:, ts(st, 128)], id_t[:])
                            nc.vector.tensor_copy(v_sd[:, st, :], pvt[:])
                    else:
                        # rotary in place; half-swap via cross-partition DVE copies
                        sl = ts(sc, SCW)
                        qsw = rtmp.tile([128, SCW], f16, tag="qsw", name=f"qsw{sc}_{ct}")
                        nc.vector.tensor_copy(qsw[0:64, :], qcb[ct][64:128, sl])
                        nc.vector.tensor_copy(qsw[64:128, :], qcb[ct][0:64, sl])
                        m1 = rtmp.tile([128, SCW], f16, tag="rtmp", name=f"m1_{sc}_{ct}")
                        nc.vector.tensor_mul(m1[:], qcb[ct][:, sl], cos_t[:, sl])
                        m2 = rtmp.tile([128, SCW], f16, tag="rtmp", name=f"m2_{sc}_{ct}")
                        nc.vector.tensor_mul(m2[:], qsw[:], sin_t[:, sl])
                        nc.vector.tensor_add(qcb[ct][:, sl], m1[:], m2[:])

            attn_state = {}

            def attn_prep(qc):
                # k-tile order: fully-causal ("old") tiles first, the 4
                # diagonal tiles last; diagonal tile ja only covers
                # q >= 128*ja of the chunk (causal trim).
                nkt = 4 * (qc + 1)
                kt_order = list(range(nkt - 4)) + list(range(nkt - 4, nkt))
                pairs = [(kt_order[2 * j], kt_order[2 * j + 1]) for j in range(nkt // 2)]
                nop = 2 * qc                  # number of old (full) pairs
                flat = [(h, j) for h in range(HL) for j in range(len(pairs))]
                ets = {}
                den_acc = {}
                den_m = {}

                def lo_of(kt):
                    return 128 * (kt - (nkt - 4)) if kt >= nkt - 4 else 0

                def scores_pair(h, j):
                    ka, kb = pairs[j]
                    scps = psS.tile([128, 2, SCW], fp32, tag="sc", name=f"sc{h}_{qc}_{j}")
                    for i, kt in ((0, ka), (1, kb)):
                        lo = lo_of(kt)
                        nc.tensor.matmul(
                            scps[:, i, lo:SCW], qcb[4][:, ts(kt, 128)],
                            qcb[h][:, qc * SCW + lo : (qc + 1) * SCW],
                            start=True, stop=True,
                        )
                    et = epool.tile([128, 2, SCW], f16, tag="exp", name=f"e{h}_{qc}_{j}")
                    lo_a, lo_b = lo_of(ka), lo_of(kb)
                    if lo_a == lo_b:
                        nc.scalar.activation(
                            et[:, :, lo_a:SCW], scps[:, :, lo_a:SCW],
                            mybir.ActivationFunctionType.Exp,
                            bias=zb_t[:, 0:1], scale=SCALE,
                        )
                    else:
                        for i, lo in ((0, lo_a), (1, lo_b)):
                            nc.scalar.activation(
                                et[:, i, lo:SCW], scps[:, i, lo:SCW],
                                mybir.ActivationFunctionType.Exp,
                                bias=zb_t[:, 0:1], scale=SCALE,
                            )
                    for i, kt in ((0, ka), (1, kb)):
                        if kt >= nkt - 4:
                            ja = kt - (nkt - 4)
                            sl = slice(128 * ja, 128 * ja + 128)
                            nc.vector.tensor_mul(et[:, i, sl], et[:, i, sl], tri_t[:])
                    if j < nop:
                        # pre-sum fully-causal tiles for the denominator
                        if j == 0:
                            den_acc[h] = et
                        else:
                            na = denpool.tile(
                                [128, 2, SCW], f16, tag="den", name=f"d{h}_{qc}_{j}"
                            )
                            nc.vector.tensor_add(na[:], den_acc[h][:], et[:])
                            den_acc[h] = na
                        if j == nop - 1:
                            dm = dmpool.tile([128, SCW], f16, tag="dm", name=f"dm{h}_{qc}")
                            nc.vector.tensor_add(
                                dm[:], den_acc[h][:, 0, :], den_acc[h][:, 1, :]
                            )
                            den_m[h] = dm
                    ets[h, j] = et

                return dict(
                    pairs=pairs, flat=flat, ets=ets, scores_pair=scores_pair,
                    lo_of=lo_of, nop=nop, den_m=den_m,
                )

            def attn_prefill(qc):
                st = attn_state[qc] = attn_prep(qc)
                for idx in range(min(LAP, len(st["flat"]))):
                    st["scores_pair"](*st["flat"][idx])

            def attn_body(qc):
                st = attn_state.pop(qc)
                pairs, flat, ets, scores_pair, lo_of, nop, den_m = (
                    st["pairs"], st["flat"], st["ets"], st["scores_pair"],
                    st["lo_of"], st["nop"], st["den_m"])
                npair = len(pairs)
                cps = {}
                sps = {}
                for idx, (h, j) in enumerate(flat):
                    if idx + LAP < len(flat):
                        scores_pair(*flat[idx + LAP])
                    if j == 0:
                        cps[h] = psC.tile([128, SCW], fp32, tag="ctx", name=f"c{h}_{qc}")
                        sps[h] = psMM.tile([128, SCW], fp32, tag="mm", name=f"s{h}_{qc}")
                    ka, kb = pairs[j]
                    et = ets.pop((h, j))
                    for i, kt in ((0, ka), (1, kb)):
                        lo = lo_of(kt)
                        first = (j == 0 and i == 0)
                        last = (j == npair - 1 and i == 1)
                        nc.tensor.matmul(
                            cps[h][:, lo:SCW], v_sd[:, kt, :], et[:, i, lo:SCW],
                            start=first, stop=last,
                        )
                    if j >= npair - 2:
                        # denominator column-sums: merged old tiles once, then
                        # the 4 trimmed diagonal tiles.
                        if j == npair - 2:
                            if nop > 0:
                                nc.tensor.matmul(
                                    sps[h][:], ones_t[:], den_m[h][:],
                                    start=True, stop=False,
                                )
                            for i, kt in ((0, ka), (1, kb)):
                                ja = kt - (npair * 2 - 4)
                                lo = 128 * ja
                                nc.tensor.matmul(
                                    sps[h][:, lo:SCW], ones_t[:], et[:, i, lo:SCW],
                                    start=(nop == 0 and ja == 0), stop=False,
                                )
                        else:
                            for i, kt in ((0, ka), (1, kb)):
                                ja = kt - (npair * 2 - 4)
                                lo = 128 * ja
                                nc.tensor.matmul(
                                    sps[h][:, lo:SCW], ones_t[:], et[:, i, lo:SCW],
                                    start=False, stop=(ja == 3),
                                )
                    if j == npair - 1:
                        # all sps rows are identical -> full-tile reciprocal,
                        # no partition broadcast needed
                        rec = rpool.tile([128, SCW], fp32, tag="rec", name=f"r{h}_{qc}")
                        nc.vector.reciprocal_approx_fast(rec[:], sps[h][:])
                        nc.vector.tensor_mul(
                            ctxT[h][:, ts(qc, SCW)], cps[h][:], rec[:]
                        )

            def outproj_chunk(qc):
                dma_eng = nc.sync if qc == NSC - 1 else nc.gpsimd
                for sti in range(4):
                    st = qc * 4 + sti
                    for ec in range(NSC):
                        po = psC.tile([128, SCW], fp32, tag="ctx", name=f"o{st}_{ec}")
                        for h in range(HL):
                            nc.tensor.matmul(
                                po[:],
                                ctxT[h][:, ts(st, 128)],
                                wout_t[:, h, ts(ec, SCW)],
                                start=(h == 0), stop=(h == HL - 1),
                            )
                        ob = opool.tile([128, SCW], f16, tag="ob", name=f"ob{st}_{ec}")
                        nc.scalar.copy(ob[:], po[:])
                        dma_eng.dma_start(out_p[ts(st, 128), ts(ec, SCW)], ob[:])

            # ---- fused main loop, attention one chunk behind the GEMM:
            # conv/rot DVE work for chunk sc hides under attention(sc-1) PE work
            for sc in range(NSC):
                if sc + 1 < NSC:
                    load_xt(sc + 1)
                if sc > 0:
                    attn_prefill(sc - 1)
                gemm_chunk(sc)
                if sc > 0:
                    attn_body(sc - 1)
                    outproj_chunk(sc - 1)
                conv_rot_chunk(sc)
            attn_prefill(NSC - 1)
            attn_body(NSC - 1)
            outproj_chunk(NSC - 1)

    nc.compile()
    return nc


def _host_prep():
    """Precompute per-core-independent constant arrays."""
    inv_freq = 1.0 / (ROT_BASE ** (np.arange(0, D, 2, dtype=np.float32) / D))
    t = np.arange(S, dtype=np.float32)
    freqs = np.outer(t, inv_freq)                       # [S, 64]
    cos = np.cos(freqs).T                               # [64, S]
    sin = np.sin(freqs).T
    cos2 = np.concatenate([cos, cos], axis=0).astype(F16)     # [128, S]
    sin2 = np.concatenate([-sin, sin], axis=0).astype(F16)
    # tri[k, q] = 1 where k <= q: within-tile causal triangle
    tri = np.triu(np.ones((128, 128), np.float32)).astype(F16)
    ident = np.eye(128, dtype=np.float32).astype(F16)
    return cos2, sin2, tri, ident


def _shard_inputs(x, W_in, b_in, conv_w, conv_b, W_out):
    cos2, sin2, tri, ident = _host_prep()
    xT = [np.ascontiguousarray(np.asarray(x[b]).T).astype(F16) for b in range(B)]
    in_maps = []
    for core in range(N_CORES):
        b, g = divmod(core, 4)
        qcols = slice(g * HL * D, (g + 1) * HL * D)
        kcols = slice(H * D + g * D, H * D + (g + 1) * D)
        vcols = slice(H * D + HKV * D + g * D, H * D + HKV * D + (g + 1) * D)
        csel = np.r_[qcols, kcols, vcols]               # 768 channel indices
        win_s = np.ascontiguousarray(
            W_in[:, csel].reshape(NEO, 128, NCT, 128).transpose(2, 1, 0, 3)
        ).astype(F16)                                              # [6, 128, 16, 128]
        binv_s = np.ascontiguousarray(
            b_in[csel].reshape(NCT, 128).T).astype(np.float32)     # [128, 6]
        convw_s = np.ascontiguousarray(
            conv_w[csel].reshape(NCT, 128, DCONV).transpose(1, 0, 2)
        ).astype(np.float32)                                       # [128, 6, 4]
        convb_s = np.ascontiguousarray(
            conv_b[csel].reshape(NCT, 128).T).astype(np.float32)
        wout_s = np.ascontiguousarray(
            W_out[g * HL * D : (g + 1) * HL * D, :]).astype(F16)   # [512, E]
        in_maps.append({
            "xT": xT[b],
            "win": win_s,
            "wout": wout_s,
            "binv": binv_s,
            "convw": convw_s,
            "convb": convb_s,
            "cos2": cos2,
            "sin2": sin2,
            "tri": tri,
            "ident": ident,
        })
    return in_maps


def _get_nc():
    if "nc" not in _cache:
        _cache["nc"] = _build_program()
    return _cache["nc"]


def run(x, W_in, b_in, conv_w, conv_b, W_out, b_out, trace=False, **rb_kwargs):
    from concourse import bass_utils

    x = np.asarray(x, dtype=np.float32)
    W_in = np.asarray(W_in, dtype=np.float32)
    b_in = np.asarray(b_in, dtype=np.float32)
    conv_w = np.asarray(conv_w, dtype=np.float32)
    conv_b = np.asarray(conv_b, dtype=np.float32)
    W_out = np.asarray(W_out, dtype=np.float32)
    b_out = np.asarray(b_out, dtype=np.float32)

    nc = _get_nc()
    in_maps = _shard_inputs(x, W_in, b_in, conv_w, conv_b, W_out)
    res = bass_utils.run_bass_kernel_spmd(
        nc, in_maps, core_ids=list(range(N_CORES)), trace=trace, **rb_kwargs
    )
    partial = [res.results[c]["out_p"] for c in range(N_CORES)]
    out = np.empty((B, S, E), dtype=np.float32)
    for b in range(B):
        acc = partial[4 * b].astype(np.float32)
        for g in range(1, 4):
            acc = acc + partial[4 * b + g]
        out[b] = acc + b_out
    return out, res


def kernel(x, W_in, b_in, conv_w, conv_b, W_out, b_out):
    out, _ = run(x, W_in, b_in, conv_w, conv_b, W_out, b_out, trace=False)
    return out


# revision 17
# speedup vs baseline: 1.1155x; 1.0439x over previous
"""Trainium2 Bass kernel for GQA MHA with causal depthwise conv + rotary.

Sharding: 8 cores = 2 batches x 4 head-groups. Each core (b, g) computes
q heads 4g..4g+3 and kv head g for batch b (tensor-parallel over heads,
data-parallel over batch; GQA repeat stays core-local). The out-projection
is row-sharded over head groups, producing partial [S, E] sums per core
that are reduced on the host during unshard, plus b_out.

Device layout choices:
  - qkv computed in [c, s] layout (channels on partitions) so the depthwise
    conv along s is a free-dim shifted-window op and rotary is elementwise.
  - fp16 everywhere on the 16-bit path (same PE/DVE speed as bf16, 8x the
    mantissa); fp32 PSUM accumulate.
  - conv reads come from two per-chunk ring buffers (pad_e for taps 0/2,
    pad_o, stored shifted by one, for taps 1/3) so every DVE operand is
    4B-aligned and the fp16 2x perf mode engages.
  - attention uses the "scores transposed" layout: scoresT[k, q] tiles from
    matmul(lhsT=kT, rhs=qT); exp on ACT; ctxT[d, q] = v_sd.T @ expT. No max
    subtraction is needed: logits here are O(0.1), exp cannot overflow.
  - causal trim: for the 4 diagonal k-tiles of each q-chunk the scores/ctx/
    denominator matmuls only cover q >= k-tile start; the within-tile
    triangle is a single [128,128] mask multiply per diagonal tile.
  - softmax denominator: old (fully-causal) exp tiles are pre-summed on the
    DVE, so the ones-matmul column reduction contracts 1 merged tile + 4
    trimmed diagonal tiles instead of all k-tiles. The reciprocal runs on
    the full [128, 512] PSUM tile (all rows identical), so no partition
    broadcast is needed.
"""

import numpy as np
import ml_dtypes

E = 2048
H = 16
HKV = 4
D = 128
DCONV = 4
ROT_BASE = 10000.0
B, S = 2, 2048
QKV_DIM = D * (H + 2 * HKV)   # 3072
N_CORES = 8
HL = 4                         # local q heads per core
CL = (HL + 2) * D              # 768 local qkv channels
NCT = CL // 128                # 6 local c-tiles (4 q heads, 1 k, 1 v)
SCW = 512                      # s-chunk width
NSC = S // SCW                 # 4
NEO = E // 128                 # 16 contraction chunks for the input GEMM
NST = S // 128                 # 16 s-tiles
F16 = np.float16
SCALE = 1.0 / float(np.sqrt(D))
PADW = 516                     # per-chunk tap ring width (halo + 512, even stride)

_cache: dict = {}
DEBUG_DUMP = False


def _build_program():
    import concourse.bacc as bacc
    import concourse.tile as tile
    import concourse.mybir as mybir
    from concourse.bass import ts

    fp32 = mybir.dt.float32
    f16 = mybir.dt.float16

    nc = bacc.Bacc("TRN2", target_bir_lowering=False, debug=False)

    # ---- device I/O ----
    xch = nc.dram_tensor("xch", [NSC, 128, NEO, SCW], f16, kind="ExternalInput")
    win = nc.dram_tensor("win", [NCT, 128, NEO, 128], f16, kind="ExternalInput")
    wout = nc.dram_tensor("wout", [HL * D, E], f16, kind="ExternalInput")
    convw = nc.dram_tensor("convw", [128, NCT, DCONV], fp32, kind="ExternalInput")
    convb4 = nc.dram_tensor("convb4", [128, NCT, DCONV], fp32, kind="ExternalInput")
    cos2 = nc.dram_tensor("cos2", [128, S], f16, kind="ExternalInput")
    sin2 = nc.dram_tensor("sin2", [128, S], f16, kind="ExternalInput")
    tri = nc.dram_tensor("tri", [128, 128], f16, kind="ExternalInput")
    ident = nc.dram_tensor("ident", [128, 128], f16, kind="ExternalInput")
    out_p = nc.dram_tensor("out_p", [S, E], f16, kind="ExternalOutput")
    if DEBUG_DUMP:
        qcb_dbg = nc.dram_tensor("qcb_dbg", [NCT, 128, S], f16, kind="ExternalOutput")
        v_dbg = nc.dram_tensor("v_dbg", [128, NST, 128], f16, kind="ExternalOutput")
        et_dbg = nc.dram_tensor("et_dbg", [2, 128, 2, SCW], f16, kind="ExternalOutput")

    CONV_ORDER = (4, 0, 5, 1, 2, 3)   # k, q0, v first: attention starts early
    LAP = 2                           # score-pipeline lookahead (pairs)

    with tile.TileContext(nc) as tc:
        with (
            tc.tile_pool(name="const", bufs=1) as cpool,
            tc.tile_pool(name="xt", bufs=2) as xpool,
            tc.tile_pool(name="qkvpad", bufs=1) as padpool,
            tc.tile_pool(name="ctmp", bufs=2) as ctmp,
            tc.tile_pool(name="rtmp", bufs=2) as rtmp,
            tc.tile_pool(name="qk", bufs=NCT) as qkpool,
            tc.tile_pool(name="vsd", bufs=1) as vpool,
            tc.tile_pool(name="exp", bufs=6) as epool,
            tc.tile_pool(name="den", bufs=2) as denpool,
            tc.tile_pool(name="dm", bufs=2) as dmpool,
            tc.tile_pool(name="ctx", bufs=HL) as ctxpool,
            tc.tile_pool(name="rec", bufs=1) as rpool,
            tc.tile_pool(name="outsb", bufs=2) as opool,
            tc.tile_pool(name="psS", bufs=2, space="PSUM") as psS,
            tc.tile_pool(name="psMM", bufs=2, space="PSUM") as psMM,
            tc.tile_pool(name="psC", bufs=2, space="PSUM") as psC,
        ):
            # ---- tiles + fine-grained startup DMA (first GEMM tile ASAP) ----
            win_t = cpool.tile([128, NEO, CL], f16)
            xt_tiles = [None] * NSC
            xt0 = xpool.tile([128, NEO, SCW], f16, tag="xt", name="xt0")
            ct0 = CONV_ORDER[0]

            # priority: exactly what the first matmuls (sc=0, ct0) need, in
            # eo-pair steps, split across the sync and gpsimd DMA issuers.
            nc.sync.dma_start(win_t[:, 0:2, ts(ct0, 128)], win[ct0, :, 0:2, :])
            nc.gpsimd.dma_start(xt0[:, 0:2, :], xch[0, :, 0:2, :])
            nc.sync.dma_start(win_t[:, 2:4, ts(ct0, 128)], win[ct0, :, 2:4, :])
            nc.gpsimd.dma_start(xt0[:, 2:4, :], xch[0, :, 2:4, :])
            nc.sync.dma_start(win_t[:, 4:16, ts(ct0, 128)], win[ct0, :, 4:16, :])
            nc.gpsimd.dma_start(xt0[:, 4:8, :], xch[0, :, 4:8, :])
            nc.sync.dma_start(xt0[:, 8:12, :], xch[0, :, 8:12, :])
            nc.gpsimd.dma_start(xt0[:, 12:16, :], xch[0, :, 12:16, :])
            xt_tiles[0] = xt0

            convw_t = cpool.tile([128, NCT, DCONV], fp32)
            nc.gpsimd.dma_start(convw_t[:], convw[:])
            cb4_t = cpool.tile([128, NCT, DCONV], fp32)
            nc.gpsimd.dma_start(cb4_t[:], convb4[:])
            # remaining GEMM weights in consumption order, alternating issuers
            nc.sync.dma_start(win_t[:, :, ts(0, 128)], win[0])
            cos_t = cpool.tile([128, S], f16)
            nc.gpsimd.dma_start(cos_t[:], cos2[:])
            sin_t = cpool.tile([128, S], f16)
            nc.gpsimd.dma_start(sin_t[:], sin2[:])
            nc.sync.dma_start(win_t[:, :, ts(5, 128)], win[5])
            id_t = cpool.tile([128, 128], f16)
            nc.gpsimd.dma_start(id_t[:], ident[:])
            tri_t = cpool.tile([128, 128], f16)
            nc.gpsimd.dma_start(tri_t[:], tri[:])
            nc.sync.dma_start(win_t[:, :, ts(1, 128)], win[1])
            nc.gpsimd.dma_start(win_t[:, :, ts(2, 128)], win[2])
            nc.sync.dma_start(win_t[:, :, ts(3, 128)], win[3])
            wout_t = cpool.tile([128, HL, E], f16)
            nc.gpsimd.dma_start(
                wout_t[:], wout[:].rearrange("(co p) e -> p co e", p=128)
            )

            ones_t = cpool.tile([128, 128], f16)
            nc.vector.memset(ones_t[:], 1.0)
            zb_t = cpool.tile([128, 1], fp32)
            nc.vector.memset(zb_t[:], 0.0)

            def load_xt(sc):
                xt = xpool.tile([128, NEO, SCW], f16, tag="xt", name=f"xt{sc}")
                for qtr in range(4):
                    nc.sync.dma_start(
                        xt[:, ts(qtr, 4), :], xch[sc, :, ts(qtr, 4), :]
                    )
                xt_tiles[sc] = xt

            # per-chunk conv tap rings: tb[k][i] = convw_k * raw[s0-3+k+i]
            # (+ bias), written pre-scaled by the ACT epilogue so the conv on
            # DVE is just 3 aligned fp16 tensor_tensor adds in 2x mode.
            tb = [
                padpool.tile([128, NCT, 2, PADW], f16, name=f"tb{k}")
                for k in range(DCONV)
            ]
            for k in range(DCONV - 1):
                nc.vector.memset(tb[k][:, :, 0, 0 : 3 - k], 0.0)

            qcb = [None] * NCT
            for ct in range(NCT):
                qcb[ct] = qkpool.tile([128, S], f16, tag="qcb", name=f"qcb{ct}")
            v_sd = vpool.tile([128, NST, 128], f16)
            ctxT = [None] * HL
            for h in range(HL):
                ctxT[h] = ctxpool.tile([128, S], f16, tag="ctxT", name=f"ctxT{h}")

            def gemm_chunk(sc):
                xt = xt_tiles[sc]
                buf = sc % 2
                for ct in CONV_ORDER:
                    ps = psMM.tile([128, SCW], fp32, tag="mm", name=f"g{sc}_{ct}")
                    for eo in range(NEO):
                        nc.tensor.matmul(
                            ps[:],
                            win_t[:, eo, ts(ct, 128)],
                            xt[:, eo, :],
                            start=(eo == 0),
                            stop=(eo == NEO - 1),
                        )
                    for k in range(DCONV):
                        nc.scalar.activation(
                            tb[k][:, ct, buf, 3 - k : 515 - k], ps[:],
                            mybir.ActivationFunctionType.Identity,
                            bias=cb4_t[:, ct, k : k + 1],
                            scale=convw_t[:, ct, k : k + 1],
                        )

            def conv_rot_chunk(sc):
                buf = sc % 2
                for ct in CONV_ORDER:
                    if sc > 0:
                        for k in range(DCONV - 1):
                            nc.vector.tensor_copy(
                                tb[k][:, ct, buf, 0 : 3 - k],
                                tb[k][:, ct, 1 - buf, SCW : SCW + 3 - k],
                            )
                    # conv = sum of the 4 pre-scaled shifted taps (2x-mode TT)
                    s1 = ctmp.tile([128, SCW], f16, tag="ctmp", name=f"s1_{sc}_{ct}")
                    nc.vector.tensor_add(
                        s1[:], tb[0][:, ct, buf, 0:SCW], tb[1][:, ct, buf, 0:SCW]
                    )
                    s2 = ctmp.tile([128, SCW], f16, tag="ctmp", name=f"s2_{sc}_{ct}")
                    nc.vector.tensor_add(
                        s2[:], tb[2][:, ct, buf, 0:SCW], tb[3][:, ct, buf, 0:SCW]
                    )
                    nc.vector.tensor_add(qcb[ct][:, ts(sc, SCW)], s1[:], s2[:])
                    if ct == 5:
                        for sti in range(4):
                            st = 4 * sc + sti
                            pvt = psMM.tile([128, 128], f16, tag="mm", name=f"vt{st}")
                            nc.tensor.transpose(pvt[:], qcb[5][:, ts(st, 128)], id_t[:])
                            nc.vector.tensor_copy(v_sd[:, st, :], pvt[:])
                    else:
                        # rotary in place; half-swap via cross-partition DVE copies
                        sl = ts(sc, SCW)
                        qsw = rtmp.tile([128, SCW], f16, tag="qsw", name=f"qsw{sc}_{ct}")
                        nc.vector.tensor_copy(qsw[0:64, :], qcb[ct][64:128, sl])
                        nc.vector.tensor_copy(qsw[64:128, :], qcb[ct][0:64, sl])
                        m1 = rtmp.tile([128, SCW], f16, tag="rtmp", name=f"m1_{sc}_{ct}")
                        nc.vector.tensor_mul(m1[:], qcb[ct][:, sl], cos_t[:, sl])
                        m2 = rtmp.tile([128, SCW], f16, tag="rtmp", name=f"m2_{sc}_{ct}")
                        nc.vector.tensor_mul(m2[:], qsw[:], sin_t[:, sl])
                        nc.vector.tensor_add(qcb[ct][:, sl], m1[:], m2[:])

            attn_state = {}

            def attn_prep(qc):
                # k-tile order: fully-causal ("old") tiles first, the 4
                # diagonal tiles last; diagonal tile ja only covers
                # q >= 128*ja of the chunk (causal trim).
                nkt = 4 * (qc + 1)
                kt_order = list(range(nkt - 4)) + list(range(nkt - 4, nkt))
                pairs = [(kt_order[2 * j], kt_order[2 * j + 1]) for j in range(nkt // 2)]
                nop = 2 * qc                  # number of old (full) pairs
                flat = [(h, j) for h in range(HL) for j in range(len(pairs))]
                ets = {}
                den_acc = {}
                den_m = {}

                def lo_of(kt):
                    # true causal-valid start col for ctx/denominator matmuls
                    return 128 * (kt - (nkt - 4)) if kt >= nkt - 4 else 0

                def mm_lo_of(kt):
                    # scores/exp range, widened so both halves of a pair are
                    # equal-width (single exp call); extra cols are never read
                    if kt < nkt - 4:
                        return 0
                    return 256 if kt - (nkt - 4) >= 2 else 0

                def scores_pair(h, j):
                    ka, kb = pairs[j]
                    lo = mm_lo_of(ka)
                    scps = psS.tile([128, 2, SCW], fp32, tag="sc", name=f"sc{h}_{qc}_{j}")
                    for i, kt in ((0, ka), (1, kb)):
                        nc.tensor.matmul(
                            scps[:, i, lo:SCW], qcb[4][:, ts(kt, 128)],
                            qcb[h][:, qc * SCW + lo : (qc + 1) * SCW],
                            start=True, stop=True,
                        )
                    et = epool.tile([128, 2, SCW], f16, tag="exp", name=f"e{h}_{qc}_{j}")
                    nc.scalar.activation(
                        et[:, :, lo:SCW], scps[:, :, lo:SCW],
                        mybir.ActivationFunctionType.Exp,
                        bias=zb_t[:, 0:1], scale=SCALE,
                    )
                    for i, kt in ((0, ka), (1, kb)):
                        if kt >= nkt - 4:
                            ja = kt - (nkt - 4)
                            sl = slice(128 * ja, 128 * ja + 128)
                            nc.vector.tensor_mul(et[:, i, sl], et[:, i, sl], tri_t[:])
                    if j < nop:
                        # pre-sum fully-causal tiles for the denominator
                        if j == 0:
                            den_acc[h] = et
                        else:
                            na = denpool.tile(
                                [128, 2, SCW], f16, tag="den", name=f"d{h}_{qc}_{j}"
                            )
                            nc.vector.tensor_add(na[:], den_acc[h][:], et[:])
                            den_acc[h] = na
                        if j == nop - 1:
                            dm = dmpool.tile([128, SCW], f16, tag="dm", name=f"dm{h}_{qc}")
                            nc.vector.tensor_add(
                                dm[:], den_acc[h][:, 0, :], den_acc[h][:, 1, :]
                            )
                            den_m[h] = dm
                    ets[h, j] = et

                return dict(
                    pairs=pairs, flat=flat, ets=ets, scores_pair=scores_pair,
                    lo_of=lo_of, nop=nop, den_m=den_m,
                )

            def attn_prefill(qc):
                st = attn_state[qc] = attn_prep(qc)
                for idx in range(min(LAP, len(st["flat"]))):
                    st["scores_pair"](*st["flat"][idx])

            def attn_body(qc):
                st = attn_state.pop(qc)
                pairs, flat, ets, scores_pair, lo_of, nop, den_m = (
                    st["pairs"], st["flat"], st["ets"], st["scores_pair"],
                    st["lo_of"], st["nop"], st["den_m"])
                npair = len(pairs)
                cps = {}
                sps = {}
                for idx, (h, j) in enumerate(flat):
                    if idx + LAP < len(flat):
                        scores_pair(*flat[idx + LAP])
                    if j == 0:
                        cps[h] = psC.tile([128, SCW], fp32, tag="ctx", name=f"c{h}_{qc}")
                        sps[h] = psMM.tile([128, SCW], fp32, tag="mm", name=f"s{h}_{qc}")
                    ka, kb = pairs[j]
                    et = ets.pop((h, j))
                    for i, kt in ((0, ka), (1, kb)):
                        lo = lo_of(kt)
                        first = (j == 0 and i == 0)
                        last = (j == npair - 1 and i == 1)
                        nc.tensor.matmul(
                            cps[h][:, lo:SCW], v_sd[:, kt, :], et[:, i, lo:SCW],
                            start=first, stop=last,
                        )
                    if j >= npair - 2:
                        # denominator column-sums: merged old tiles once, then
                        # the 4 trimmed diagonal tiles.
                        if j == npair - 2:
                            if nop > 0:
                                nc.tensor.matmul(
                                    sps[h][:], ones_t[:], den_m[h][:],
                                    start=True, stop=False,
                                )
                            for i, kt in ((0, ka), (1, kb)):
                                ja = kt - (npair * 2 - 4)
                                lo = 128 * ja
                                nc.tensor.matmul(
                                    sps[h][:, lo:SCW], ones_t[:], et[:, i, lo:SCW],
                                    start=(nop == 0 and ja == 0), stop=False,
                                )
                        else:
                            for i, kt in ((0, ka), (1, kb)):
                                ja = kt - (npair * 2 - 4)
                                lo = 128 * ja
                                nc.tensor.matmul(
                                    sps[h][:, lo:SCW], ones_t[:], et[:, i, lo:SCW],
                                    start=False, stop=(ja == 3),
                                )
                    if j == npair - 1:
                        # all sps rows are identical -> full-tile reciprocal,
                        # no partition broadcast needed
                        rec = rpool.tile([128, SCW], fp32, tag="rec", name=f"r{h}_{qc}")
                        nc.vector.reciprocal_approx_fast(rec[:], sps[h][:])
                        nc.vector.tensor_mul(
                            ctxT[h][:, ts(qc, SCW)], cps[h][:], rec[:]
                        )

            def outproj_chunk(qc):
                last = qc == NSC - 1
                for sti in range(4):
                    st = qc * 4 + sti
                    obrow = opool.tile([128, NSC, SCW], f16, tag="ob", name=f"ob{st}")
                    for ec in range(NSC):
                        po = psC.tile([128, SCW], fp32, tag="ctx", name=f"o{st}_{ec}")
                        for h in range(HL):
                            nc.tensor.matmul(
                                po[:],
                                ctxT[h][:, ts(st, 128)],
                                wout_t[:, h, ts(ec, SCW)],
                                start=(h == 0), stop=(h == HL - 1),
                            )
                        nc.scalar.copy(obrow[:, ec, :], po[:])
                        if last:
                            # fine-grained drain on parallel queues at the tail
                            nc.sync.dma_start(
                                out_p[ts(st, 128), ts(ec, SCW)], obrow[:, ec, :]
                            )
                    if not last:
                        nc.gpsimd.dma_start(out_p[ts(st, 128), :], obrow[:])

            # ---- fused main loop, attention one chunk behind the GEMM:
            # conv/rot DVE work for chunk sc hides under attention(sc-1) PE work
            for sc in range(NSC):
                if sc + 1 < NSC:
                    load_xt(sc + 1)
                if sc > 0:
                    attn_prefill(sc - 1)
                gemm_chunk(sc)
                if sc > 0:
                    attn_body(sc - 1)
                    outproj_chunk(sc - 1)
                conv_rot_chunk(sc)
            attn_prefill(NSC - 1)
            attn_body(NSC - 1)
            outproj_chunk(NSC - 1)
            if DEBUG_DUMP:
                for ct in range(NCT):
                    nc.sync.dma_start(qcb_dbg[ct], qcb[ct][:])
                nc.sync.dma_start(v_dbg[:], v_sd[:])

    nc.compile()
    return nc


def _host_prep():
    """Precompute per-core-independent constant arrays."""
    inv_freq = 1.0 / (ROT_BASE ** (np.arange(0, D, 2, dtype=np.float32) / D))
    t = np.arange(S, dtype=np.float32)
    freqs = np.outer(t, inv_freq)                       # [S, 64]
    cos = np.cos(freqs).T                               # [64, S]
    sin = np.sin(freqs).T
    cos2 = np.concatenate([cos, cos], axis=0).astype(F16)     # [128, S]
    sin2 = np.concatenate([-sin, sin], axis=0).astype(F16)
    # tri[k, q] = 1 where k <= q: within-tile causal triangle
    tri = np.triu(np.ones((128, 128), np.float32)).astype(F16)
    ident = np.eye(128, dtype=np.float32).astype(F16)
    return cos2, sin2, tri, ident


def _shard_inputs(x, W_in, b_in, conv_w, conv_b, W_out):
    cos2, sin2, tri, ident = _host_prep()
    # chunk-major x layout: per partition, each chunk's 16 eo-rows are
    # contiguous (16 KB) -> large DMA descriptors
    xch = [
        np.ascontiguousarray(
            np.asarray(x[b]).T.reshape(NEO, 128, NSC, SCW).transpose(2, 1, 0, 3)
        ).astype(F16)
        for b in range(B)
    ]
    in_maps = []
    for core in range(N_CORES):
        b, g = divmod(core, 4)
        qcols = slice(g * HL * D, (g + 1) * HL * D)
        kcols = slice(H * D + g * D, H * D + (g + 1) * D)
        vcols = slice(H * D + HKV * D + g * D, H * D + HKV * D + (g + 1) * D)
        csel = np.r_[qcols, kcols, vcols]               # 768 channel indices
        win_s = np.ascontiguousarray(
            W_in[:, csel].reshape(NEO, 128, NCT, 128).transpose(2, 1, 0, 3)
        ).astype(F16)                                              # [6, 128, 16, 128]
        convw_s = np.ascontiguousarray(
            conv_w[csel].reshape(NCT, 128, DCONV).transpose(1, 0, 2)
        ).astype(np.float32)                                       # [128, 6, 4]
        # per-tap epilogue bias: w_k * b_in, plus conv_b on tap 3 (the only
        # tap with no zero-halo cells, so every position keeps the bias)
        cb4 = conv_w[csel] * b_in[csel][:, None]                   # [768, 4]
        cb4[:, 3] += conv_b[csel]
        cb4_s = np.ascontiguousarray(
            cb4.reshape(NCT, 128, DCONV).transpose(1, 0, 2)
        ).astype(np.float32)                                       # [128, 6, 4]
        wout_s = np.ascontiguousarray(
            W_out[g * HL * D : (g + 1) * HL * D, :]).astype(F16)   # [512, E]
        in_maps.append({
            "xch": xch[b],
            "win": win_s,
            "wout": wout_s,
            "convw": convw_s,
            "convb4": cb4_s,
            "cos2": cos2,
            "sin2": sin2,
            "tri": tri,
            "ident": ident,
        })
    return in_maps


def _get_nc():
    if "nc" not in _cache:
        _cache["nc"] = _build_program()
    return _cache["nc"]


def run(x, W_in, b_in, conv_w, conv_b, W_out, b_out, trace=False, **rb_kwargs):
    from concourse import bass_utils

    x = np.asarray(x, dtype=np.float32)
    W_in = np.asarray(W_in, dtype=np.float32)
    b_in = np.asarray(b_in, dtype=np.float32)
    conv_w = np.asarray(conv_w, dtype=np.float32)
    conv_b = np.asarray(conv_b, dtype=np.float32)
    W_out = np.asarray(W_out, dtype=np.float32)
    b_out = np.asarray(b_out, dtype=np.float32)

    nc = _get_nc()
    in_maps = _shard_inputs(x, W_in, b_in, conv_w, conv_b, W_out)
    res = bass_utils.run_bass_kernel_spmd(
        nc, in_maps, core_ids=list(range(N_CORES)), trace=trace, **rb_kwargs
    )
    partial = [res.results[c]["out_p"] for c in range(N_CORES)]
    out = np.empty((B, S, E), dtype=np.float32)
    for b in range(B):
        acc = partial[4 * b].astype(np.float32)
        for g in range(1, 4):
            acc = acc + partial[4 * b + g]
        out[b] = acc + b_out
    return out, res


def kernel(x, W_in, b_in, conv_w, conv_b, W_out, b_out):
    out, _ = run(x, W_in, b_in, conv_w, conv_b, W_out, b_out, trace=False)
    return out
